# revision 6
# baseline (speedup 1.0000x reference)
"""MultiHuberLoss Trainium2 kernel (bf16 stream + fused accumulation).

Reference (per element, with m = +x at the target class, -x elsewhere):
    hinge = max(0, 1 - m);  loss = where(m >= -1, hinge^2, -4m);  out = sum(loss)/N

Math (exact identities), treating every element as non-target (m = -x):
    G(x) = (v+1)^2 + 4*(u-1),  v = clamp(x,-1,1), u = max(x,1)
Per-row correction for the target column t:  G_true - G(x_t) = -4*x_t.

Host-side prep (pure layout / precision, no loss arithmetic):
  - cast input to bf16 (tolerance is 2e-2; measured pipeline err ~1.5e-6)
  - swap each row's target element into column 0 (the per-row loss is a
    sum over the row's values, so an intra-row permutation is invariant);
    the correction then reads a strided slice instead of a gather.

Device per core (8192 rows = [128 partitions, 64000 free] bf16):
  - stream 8 tiles of [128, 8000] (2 MB DMAs)
  - DVE tensor_scalar (4x mode): v = clamp(x,-1,1); u4 = 4*max(x,1) with
    fused per-partition accumulation (accum_out)
  - squares split ACT/DVE to balance engines: ACT Square(v+1) with accum
    on the first N_ACT columns, DVE scalar_tensor_tensor (v+2)*v with
    accum on the rest ((v+1)^2 = (v^2+2v) + 1, count folded into bias)
  - correction: -4 * x[:, j*1000] strided slice, DVE accum
  - final: one free-dim reduce, PE ones-matmul partition reduce, scale by
    1/N with the constant terms folded into the activation bias
"""

import ml_dtypes
import numpy as np

import concourse.bacc as bacc
import concourse.mybir as mybir
from concourse.bass_utils import run_bass_kernel_spmd
from concourse.tile import TileContext

N_TOTAL = 65536
C = 1000
N_CORES = 8
ROWS = N_TOTAL // N_CORES  # 8192 rows per core
P = 128                    # partitions
JPP = ROWS // P            # 64 rows per partition
FREE = JPP * C             # 64000 bf16 per partition

TILE_FDS = [8000] * 8
assert sum(TILE_FDS) == FREE
# ACT-square columns per tile (rest squared on DVE via STT)
N_ACT = [6400] * 8

f32 = mybir.dt.float32
bf16 = mybir.dt.bfloat16
Alu = mybir.AluOpType
AF = mybir.ActivationFunctionType

NT = len(TILE_FDS)


def build_program():
    nc = bacc.Bacc(
        "TRN2", target_bir_lowering=False, debug=False, num_devices=N_CORES
    )
    x = nc.dram_tensor("x", [ROWS, C], bf16, kind="ExternalInput")
    out = nc.dram_tensor("out", [1, 1], f32, kind="ExternalOutput")

    x_flat = x.ap().rearrange("(p j) c -> p (j c)", p=P)  # [128, 64000]

    n_stt_pp = sum(fd - na for fd, na in zip(TILE_FDS, N_ACT))
    # per-partition constant: +count for the STT region, -4 per element
    bias_c = (P * (n_stt_pp - 4.0 * FREE)) / N_TOTAL

    with TileContext(nc) as tc:
        with (
            tc.tile_pool(name="xp", bufs=4) as xp,
            tc.tile_pool(name="vp", bufs=2) as vp,
            tc.tile_pool(name="scr", bufs=1) as scr,
            tc.tile_pool(name="small", bufs=1) as small,
            tc.tile_pool(name="psp", bufs=1, space="PSUM") as psp,
        ):
            max_fd = max(TILE_FDS)
            u4_scr = scr.tile([P, max_fd], bf16, tag="u4_scr")
            sq_scr = scr.tile([P, max(N_ACT)], bf16, tag="sq_scr")
            stt_scr = scr.tile(
                [P, max(fd - na for fd, na in zip(TILE_FDS, N_ACT))],
                bf16, tag="stt_scr",
            )
            c0_scr = scr.tile([P, 8], f32, tag="c0_scr")
            # acc columns: [0:NT) u4, [NT:2NT) ACT sq, [2NT:3NT) STT sq,
            # [3NT:4NT) col0 correction
            acc = small.tile([P, 4 * NT], f32, tag="acc")
            nc.vector.memset(acc[:], 0.0)
            ones_f = small.tile([P, 1], f32, tag="ones_f")
            nc.vector.memset(ones_f[:], 1.0)

            off = 0
            for t, fd in enumerate(TILE_FDS):
                na = N_ACT[t]
                xt = xp.tile([P, fd], bf16)
                nc.sync.dma_start(out=xt[:], in_=x_flat[:, off:off + fd])
                v = vp.tile([P, fd], bf16)
                # v = clamp(x, -1, 1)   (4x mode)
                nc.vector.tensor_scalar(
                    v[:], xt[:], -1.0, 1.0, Alu.max, Alu.min
                )
                # ACT: sum (v+1)^2 over first na columns
                nc.scalar.activation(
                    sq_scr[:, 0:na], v[:, 0:na], AF.Square,
                    bias=1.0, scale=1.0,
                    accum_out=acc[:, NT + t:NT + t + 1],
                )
                # u = max(x, 1), reduce-add into acc col t  (x4 later)
                nc.vector.tensor_scalar(
                    u4_scr[:, 0:fd], xt[:], 1.0, 0.0, Alu.max, Alu.add,
                    accum_out=acc[:, t:t + 1],
                )
                # correction: -4 * x[:, j*C] for the rows in this tile
                ncol = fd // C
                x3 = xt[:].rearrange("p (j c) -> p j c", c=C)
                nc.vector.tensor_scalar(
                    c0_scr[:, 0:ncol],
                    x3[:, :, 0:1].squeeze(2),
                    -4.0, 0.0, Alu.mult, Alu.add,
                    accum_out=acc[:, 3 * NT + t:3 * NT + t + 1],
                )
                # DVE: sum (v^2 + 2v) over the rest  (2x mode)
                if na < fd:
                    nc.vector.scalar_tensor_tensor(
                        out=stt_scr[:, 0:fd - na],
                        in0=v[:, na:fd], scalar=2.0, in1=v[:, na:fd],
                        op0=Alu.add, op1=Alu.mult,
                        accum_out=acc[:, 2 * NT + t:2 * NT + t + 1],
                    )
                off += fd

            # ---- final combine ----
            # scale the u-columns by 4 (B term = 4*sum(u) - 4*count)
            nc.vector.tensor_scalar(
                acc[:, 0:NT], acc[:, 0:NT], 4.0, None, Alu.mult
            )
            s_p = small.tile([P, 1], f32, tag="s_p")
            nc.vector.reduce_sum(s_p, acc[:], axis=mybir.AxisListType.X)
            psS = psp.tile([1, 8], f32, tag="psS")
            nc.tensor.matmul(
                out=psS[:, 0:1], lhsT=ones_f[:], rhs=s_p[:],
                start=True, stop=True,
            )
            bias_t = small.tile([1, 1], f32, tag="bias")
            nc.vector.memset(bias_t[:], bias_c)
            res = small.tile([1, 1], f32, tag="res")
            nc.scalar.activation(
                res[:], psS[:, 0:1], AF.Identity,
                bias=bias_t[:], scale=1.0 / N_TOTAL,
            )
            nc.sync.dma_start(out=out.ap(), in_=res[:])

    nc.compile()
    return nc


_NC_CACHE = None
LAST_RESULTS = None


def kernel(input, target):
    global _NC_CACHE, LAST_RESULTS
    x = np.asarray(input)
    tg = np.asarray(target).astype(np.int64)
    assert x.shape == (N_TOTAL, C), x.shape
    assert tg.shape == (N_TOTAL,), tg.shape

    if _NC_CACHE is None:
        _NC_CACHE = build_program()
    nc = _NC_CACHE

    # bf16 cast + swap each row's target value into column 0
    xb = x.astype(ml_dtypes.bfloat16)
    rows = np.arange(N_TOTAL)
    tv = xb[rows, tg].copy()
    xb[rows, tg] = xb[rows, 0]
    xb[rows, 0] = tv

    in_maps = [
        {"x": xb[c * ROWS:(c + 1) * ROWS]}
        for c in range(N_CORES)
    ]
    res = run_bass_kernel_spmd(nc, in_maps, core_ids=list(range(N_CORES)))
    LAST_RESULTS = res
    total = np.float32(0.0)
    for r in res.results:
        total += np.float32(r["out"].reshape(()))
    return np.asarray(total, dtype=np.float32)


if __name__ == "__main__":
    rng = np.random.default_rng(0)
    xs = rng.standard_normal((N_TOTAL, C), dtype=np.float32)
    ts = rng.integers(0, C, size=(N_TOTAL,)).astype(np.int64)
    got = kernel(xs, ts)
    m = np.where(np.arange(C)[None, :] == ts[:, None], xs, -xs)
    hinge = np.maximum(0.0, 1.0 - m)
    loss = np.where(m >= -1.0, hinge * hinge, -4.0 * m)
    want = loss.sum(dtype=np.float64) / N_TOTAL
    print("got", got, "want", want, "rel", abs(got - want) / abs(want))
